# revision 7
# baseline (speedup 1.0000x reference)
"""Multi-head attention (B=4, T=2048, C=768, H=12) on 8 trn2 cores.

Sharding: core c -> (batch b = c//2, head-group g = c%2 of 6 heads).
Each core computes qkv projection for its heads, full attention over its
2048-token sequence, and a partial output projection over its heads'
feature columns. Host sums the two partial projections per batch and adds
proj_b.

Precision: logit path (qkv for q/k, scores) runs in float32r (~12-bit
mantissa matmul, full PE rate at N>=256); attention probabilities are
stored bf16; v / attn@v / proj run in bf16-or-f32r mixes. End-to-end
rel-l2 error vs fp32 reference ~2.5e-3 (measured numerically).
"""

import math
import numpy as np

import concourse.bass as bass
import concourse.bacc as bacc
import concourse.mybir as mybir
import concourse.tile as tile
from concourse.bass_utils import run_bass_kernel_spmd
from concourse.masks import make_identity

F32 = mybir.dt.float32
F32R = mybir.dt.float32r
BF16 = mybir.dt.bfloat16

T = 2048
C = 768
DH = 64
HL = 6          # heads per core
HW = HL * DH    # 384 local head-feature width
NT = T // 128   # 16 token tiles
NKC = C // 128  # 6 contraction chunks for the input dim
NPR = HL // 2   # 3 packed head pairs

# knobs
PT_COPY_ENGINE = ("vector", "scalar", "vector", "scalar")  # per transpose group


def build_nc():
    nc = bacc.Bacc(None, target_bir_lowering=False)

    xT = nc.declare_dram_parameter("xT", [C, T], F32, isOutput=False)
    wqT = nc.declare_dram_parameter("wqT", [C, HW], F32, isOutput=False)
    wkT = nc.declare_dram_parameter("wkT", [C, HW], F32, isOutput=False)
    wvT = nc.declare_dram_parameter("wvT", [C, HW], F32, isOutput=False)
    bqt = nc.declare_dram_parameter("bqt", [128, NPR], F32, isOutput=False)
    bkt = nc.declare_dram_parameter("bkt", [128, NPR], F32, isOutput=False)
    bv = nc.declare_dram_parameter("bv", [HW], F32, isOutput=False)
    projT = nc.declare_dram_parameter("projT", [HW, C], F32, isOutput=False)
    out = nc.declare_dram_parameter("out", [T, C], F32, isOutput=True)

    with tile.TileContext(nc) as tc:
        _emit(nc, tc, xT, wqT, wkT, wvT, bqt, bkt, bv, projT, out)
    nc.compile()
    return nc


def _emit(nc, tc, xT, wqT, wkT, wvT, bqt, bkt, bv, projT, out):
    from contextlib import ExitStack

    ctx = ExitStack()
    const = ctx.enter_context(tc.tile_pool(name="const", bufs=1))
    s1 = ctx.enter_context(tc.tile_pool(name="s1", bufs=1))
    persist = ctx.enter_context(tc.tile_pool(name="persist", bufs=1))
    work = ctx.enter_context(tc.tile_pool(name="work", bufs=2))
    stats = ctx.enter_context(tc.tile_pool(name="stats", bufs=4))
    psum = ctx.enter_context(tc.tile_pool(name="psum", bufs=2, space="PSUM"))

    # ---- constants ----
    ident = const.tile([128, 128], BF16, tag="ident")
    make_identity(nc, ident[:, :])
    bq_sb = const.tile([128, NPR], F32, tag="bq")
    bk_sb = const.tile([128, NPR], F32, tag="bk")
    nc.sync.dma_start(out=bq_sb[:, :], in_=bqt[:, :])
    nc.sync.dma_start(out=bk_sb[:, :], in_=bkt[:, :])
    bv_bc = const.tile([128, HW], F32, tag="bv")
    _bvap = bv[:]
    nc.sync.dma_start(
        out=bv_bc[:, :],
        in_=bass.AP(tensor=_bvap.tensor, offset=_bvap.offset,
                    ap=[[0, 128]] + list(_bvap.ap)),
    )

    # ---- stage 1: load inputs, round to f32r via staging, qkv projections ----
    xt = []
    wq = []
    wk = []
    wv = []
    for kc in range(NKC):
        stg_x = s1.tile([128, T], F32, tag="stgx", name=f"stgx{kc}", bufs=2)
        nc.sync.dma_start(out=stg_x[:, :], in_=xT[kc * 128:(kc + 1) * 128, :])
        t_x = persist.tile([128, T], F32R, tag=f"xt{kc}", name=f"xt{kc}")
        nc.vector.tensor_copy(t_x[:, :], stg_x[:, :])
        xt.append(t_x)
        for src, tag, acc in ((wqT, "wq", wq), (wkT, "wk", wk), (wvT, "wv", wv)):
            stg_w = s1.tile([128, HW], F32, tag="stgw", name=f"stg{tag}{kc}", bufs=2)
            nc.sync.dma_start(out=stg_w[:, :], in_=src[kc * 128:(kc + 1) * 128, :])
            t_w = persist.tile([128, HW], F32R, tag=f"{tag}{kc}", name=f"{tag}{kc}")
            nc.vector.tensor_copy(t_w[:, :], stg_w[:, :])
            acc.append(t_w)

    def r(t):
        return t[:, :]

    # q^T and k^T, packed two heads (128 rows) per pair tile
    qT = []
    kT = []
    for name, wlist, bias_sb, acc in (("qT", wq, bq_sb, qT), ("kT", wk, bk_sb, kT)):
        for pr in range(NPR):
            dst = persist.tile([128, T], F32R, tag=f"{name}{pr}")
            for h2 in range(2):  # halves of T
                ps = psum.tile([128, 1024], F32, tag="sc")
                for kc in range(NKC):
                    for nn in range(2):
                        n0 = (h2 * 2 + nn) * 512
                        nc.tensor.matmul(
                            ps[:, nn * 512:(nn + 1) * 512],
                            r(wlist[kc])[:, pr * 128:(pr + 1) * 128],
                            r(xt[kc])[:, n0:n0 + 512],
                            start=(kc == 0), stop=(kc == NKC - 1),
                        )
                nc.scalar.activation(
                    dst[:, h2 * 1024:(h2 + 1) * 1024], ps[:, :],
                    mybir.ActivationFunctionType.Identity,
                    bias=bias_sb[:, pr:pr + 1], scale=1.0,
                )
            acc.append(dst)

    # v in [token, feature] layout, bf16, bias added
    v_sb = []
    for tt in range(NT):
        psv = psum.tile([128, HW], F32, tag="av")
        for kc in range(NKC):
            nc.tensor.matmul(
                psv[:, :],
                r(xt[kc])[:, tt * 128:(tt + 1) * 128],
                r(wv[kc])[:, :],
                start=(kc == 0), stop=(kc == NKC - 1),
            )
        dst = persist.tile([128, HW], BF16, tag=f"v{tt}")
        nc.vector.tensor_tensor(
            out=dst[:, :], in0=psv[:, :], in1=bv_bc[:, :], op=mybir.AluOpType.add
        )
        v_sb.append(dst)

    # ---- stage 2: attention ----
    ao_sb = [
        persist.tile([128, HW], BF16, tag=f"ao{qt}", name=f"ao{qt}")
        for qt in range(NT)
    ]

    for h in range(HL):
        pr, half = divmod(h, 2)
        p0 = half * 64
        qTh = qT[pr]
        kTh = kT[pr]

        state = {}

        def emit_scores(qt):
            ps_pair = []
            for s2 in range(2):
                ps = psum.tile([128, 1024], F32, tag="sc")
                for nn in range(2):
                    n0 = (s2 * 2 + nn) * 512
                    nc.tensor.matmul(
                        ps[:, nn * 512:(nn + 1) * 512],
                        qTh[p0:p0 + 64, qt * 128:(qt + 1) * 128],
                        kTh[p0:p0 + 64, n0:n0 + 512],
                        start=True, stop=True,
                    )
                ps_pair.append(ps)
            m2 = stats.tile([128, 2], F32, tag="m2")
            nc.vector.reduce_max(m2[:, 0:1], ps_pair[0][:, :], axis=mybir.AxisListType.X)
            nc.vector.reduce_max(m2[:, 1:2], ps_pair[1][:, :], axis=mybir.AxisListType.X)
            nb8 = stats.tile([128, 1], F32, tag="nb8")
            mc = stats.tile([128, 1], F32, tag="mc")
            nc.vector.tensor_tensor(
                out=mc[:, :], in0=m2[:, 0:1], in1=m2[:, 1:2], op=mybir.AluOpType.max
            )
            nc.vector.tensor_scalar_mul(nb8[:, :], mc[:, :], -8.0)
            p_bf = work.tile([128, T], BF16, tag="pbf")
            z2 = stats.tile([128, 2], F32, tag="z2")
            for s2 in range(2):
                nc.scalar.activation(
                    p_bf[:, s2 * 1024:(s2 + 1) * 1024], ps_pair[s2][:, :],
                    mybir.ActivationFunctionType.Exp,
                    bias=nb8[:, 0:1], scale=8.0,
                    accum_out=z2[:, s2:s2 + 1],
                )
            z = stats.tile([128, 1], F32, tag="z")
            nc.vector.tensor_tensor(
                out=z[:, :], in0=z2[:, 0:1], in1=z2[:, 1:2], op=mybir.AluOpType.add
            )
            rz = stats.tile([128, 1], F32, tag="rz")
            nc.vector.reciprocal(rz[:, :], z[:, :])
            state[qt] = (p_bf, rz)

        def emit_consume(qt):
            p_bf, rz = state.pop(qt)
            pT = work.tile([128, T], BF16, tag="pT")
            for grp in range(4):
                pst = psum.tile([128, 512], BF16, tag="tr")
                for j in range(4):
                    c = grp * 4 + j
                    nc.tensor.transpose(
                        pst[:, j * 128:(j + 1) * 128],
                        p_bf[:, c * 128:(c + 1) * 128],
                        ident[:, :],
                    )
                eng = PT_COPY_ENGINE[grp]
                if eng == "vector":
                    nc.vector.tensor_copy(pT[:, grp * 512:(grp + 1) * 512], pst[:, :])
                else:
                    nc.scalar.copy(pT[:, grp * 512:(grp + 1) * 512], pst[:, :])
            pso = psum.tile([128, DH], F32, tag="av")
            for c in range(NT):
                nc.tensor.matmul(
                    pso[:, :],
                    pT[:, c * 128:(c + 1) * 128],
                    v_sb[c][:, h * DH:(h + 1) * DH],
                    start=(c == 0), stop=(c == NT - 1),
                )
            nc.scalar.activation(
                ao_sb[qt][:, h * DH:(h + 1) * DH], pso[:, :],
                mybir.ActivationFunctionType.Copy,
                bias=0.0, scale=rz[:, 0:1],
            )

        for qt in range(NT):
            emit_scores(qt)
            if qt > 0:
                emit_consume(qt - 1)
        emit_consume(NT - 1)

    # ---- stage 3: output projection (partial over local heads) ----
    pjs = []
    for c3 in range(3):
        stg = work.tile([128, C], F32, tag="pstage")
        nc.sync.dma_start(out=stg[:, :], in_=projT[c3 * 128:(c3 + 1) * 128, :])
        pj = const.tile([128, C], BF16, tag=f"pj{c3}")
        nc.vector.tensor_copy(pj[:, :], stg[:, :])
        pjs.append(pj)

    for qt in range(NT):
        pst2 = psum.tile([128, HW], BF16, tag="tr")
        for c3 in range(3):
            nc.tensor.transpose(
                pst2[:, c3 * 128:(c3 + 1) * 128],
                ao_sb[qt][:, c3 * 128:(c3 + 1) * 128],
                ident[:, :],
            )
        aoT = work.tile([128, HW], BF16, tag="aoT")
        nc.vector.tensor_copy(aoT[:, :], pst2[:, :])
        out_sb = work.tile([128, C], F32, tag="outsb")
        for n0, nsz in ((0, 512), (512, 256)):
            psp = psum.tile([128, 512], F32, tag="sc")
            for c3 in range(3):
                nc.tensor.matmul(
                    psp[:, 0:nsz],
                    aoT[:, c3 * 128:(c3 + 1) * 128],
                    pjs[c3][:, n0:n0 + nsz],
                    start=(c3 == 0), stop=(c3 == 2),
                )
            nc.scalar.copy(out_sb[:, n0:n0 + nsz], psp[:, 0:nsz])
        nc.sync.dma_start(out=out[qt * 128:(qt + 1) * 128, :], in_=out_sb[:, :])

    ctx.close()


_NC = None


def _get_nc():
    global _NC
    if _NC is None:
        _NC = build_nc()
    return _NC


def kernel(x, qkv_w, qkv_b, proj_w, proj_b):
    x = np.asarray(x, dtype=np.float32)
    qkv_w = np.asarray(qkv_w, dtype=np.float32)
    qkv_b = np.asarray(qkv_b, dtype=np.float32)
    proj_w = np.asarray(proj_w, dtype=np.float32)
    proj_b = np.asarray(proj_b, dtype=np.float32)

    B = x.shape[0]
    ins = []
    for c in range(8):
        b, g = divmod(c, 2)
        hs = slice(g * HW, (g + 1) * HW)
        wq = qkv_w[0:C][hs]
        wk = qkv_w[C:2 * C][hs]
        wv = qkv_w[2 * C:3 * C][hs]
        bq = qkv_b[0:C][hs]
        bk = qkv_b[C:2 * C][hs]
        bvv = qkv_b[2 * C:3 * C][hs]
        ins.append({
            "xT": np.ascontiguousarray(x[b].T),
            "wqT": np.ascontiguousarray(wq.T),
            "wkT": np.ascontiguousarray(wk.T),
            "wvT": np.ascontiguousarray(wv.T),
            "bqt": np.ascontiguousarray(bq.reshape(NPR, 128).T),
            "bkt": np.ascontiguousarray(bk.reshape(NPR, 128).T),
            "bv": np.ascontiguousarray(bvv),
            "projT": np.ascontiguousarray(proj_w[:, hs].T),
        })

    res = run_bass_kernel_spmd(_get_nc(), ins, list(range(8)))
    outp = np.empty((B, T, C), np.float32)
    for b in range(B):
        outp[b] = res.results[2 * b]["out"] + res.results[2 * b + 1]["out"] + proj_b
    return outp


# revision 9
# speedup vs baseline: 37.6350x; 37.6350x over previous
"""Multi-head attention (B=4, T=2048, C=768, H=12) on 8 trn2 cores.

Sharding: core c -> (batch b = c//2, head-group g = c%2 of 6 heads).
Each core computes qkv projection for its heads, full attention over its
2048-token sequence, and a partial output projection over its heads'
feature columns. Host sums the two partial projections per batch and adds
proj_b.

Precision: logit path (qkv for q/k, scores) runs in float32r (~12-bit
mantissa matmul, full PE rate at N>=256); attention probabilities are
stored bf16; v / attn@v / proj run in bf16-or-f32r mixes. End-to-end
rel-l2 error vs fp32 reference ~2.5e-3 (measured numerically).
"""

import math
import numpy as np

import concourse.bass as bass
import concourse.bacc as bacc
import concourse.mybir as mybir
import concourse.tile as tile
from concourse.bass_utils import run_bass_kernel_spmd
from concourse.masks import make_identity

F32 = mybir.dt.float32
F32R = mybir.dt.float32r
BF16 = mybir.dt.bfloat16

T = 2048
C = 768
DH = 64
HL = 6          # heads per core
HW = HL * DH    # 384 local head-feature width
NT = T // 128   # 16 token tiles
NKC = C // 128  # 6 contraction chunks for the input dim
NPR = HL // 2   # 3 packed head pairs

# knobs
PT_COPY_ENGINE = ("vector", "scalar", "vector", "scalar")  # per transpose group


def build_nc():
    nc = bacc.Bacc(None, target_bir_lowering=False)

    xT = nc.declare_dram_parameter("xT", [C, T], F32, isOutput=False)
    wqT = nc.declare_dram_parameter("wqT", [C, HW], F32, isOutput=False)
    wkT = nc.declare_dram_parameter("wkT", [C, HW], F32, isOutput=False)
    wvT = nc.declare_dram_parameter("wvT", [C, HW], F32, isOutput=False)
    bqt = nc.declare_dram_parameter("bqt", [128, NPR], F32, isOutput=False)
    bkt = nc.declare_dram_parameter("bkt", [128, NPR], F32, isOutput=False)
    bv = nc.declare_dram_parameter("bv", [HW], F32, isOutput=False)
    projT = nc.declare_dram_parameter("projT", [HW, C], F32, isOutput=False)
    out = nc.declare_dram_parameter("out", [T, C], F32, isOutput=True)

    with tile.TileContext(nc) as tc:
        _emit(nc, tc, xT, wqT, wkT, wvT, bqt, bkt, bv, projT, out)
    nc.compile()
    return nc


def _emit(nc, tc, xT, wqT, wkT, wvT, bqt, bkt, bv, projT, out):
    from contextlib import ExitStack

    ctx = ExitStack()
    const = ctx.enter_context(tc.tile_pool(name="const", bufs=1))
    s1 = ctx.enter_context(tc.tile_pool(name="s1", bufs=1))
    persist = ctx.enter_context(tc.tile_pool(name="persist", bufs=1))
    work = ctx.enter_context(tc.tile_pool(name="work", bufs=2))
    stats = ctx.enter_context(tc.tile_pool(name="stats", bufs=4))
    psum = ctx.enter_context(tc.tile_pool(name="psum", bufs=2, space="PSUM"))

    # ---- constants ----
    ident = const.tile([128, 128], BF16, tag="ident")
    make_identity(nc, ident[:, :])
    bq_sb = const.tile([128, NPR], F32, tag="bq")
    bk_sb = const.tile([128, NPR], F32, tag="bk")
    nc.sync.dma_start(out=bq_sb[:, :], in_=bqt[:, :])
    nc.sync.dma_start(out=bk_sb[:, :], in_=bkt[:, :])
    bv_bc = const.tile([128, HW], F32, tag="bv")
    _bvap = bv[:]
    nc.sync.dma_start(
        out=bv_bc[:, :],
        in_=bass.AP(tensor=_bvap.tensor, offset=_bvap.offset,
                    ap=[[0, 128]] + list(_bvap.ap)),
    )

    # ---- stage 1: load inputs, round to f32r via staging, qkv projections ----
    xt = []
    wq = []
    wk = []
    wv = []
    for kc in range(NKC):
        stg_x = s1.tile([128, T], F32, tag="stgx", name=f"stgx{kc}", bufs=2)
        nc.sync.dma_start(out=stg_x[:, :], in_=xT[kc * 128:(kc + 1) * 128, :])
        t_x = persist.tile([128, T], F32R, tag=f"xt{kc}", name=f"xt{kc}")
        nc.vector.tensor_copy(t_x[:, :], stg_x[:, :])
        xt.append(t_x)
        for src, tag, acc in ((wqT, "wq", wq), (wkT, "wk", wk), (wvT, "wv", wv)):
            stg_w = s1.tile([128, HW], F32, tag="stgw", name=f"stg{tag}{kc}", bufs=2)
            nc.sync.dma_start(out=stg_w[:, :], in_=src[kc * 128:(kc + 1) * 128, :])
            t_w = persist.tile([128, HW], F32R, tag=f"{tag}{kc}", name=f"{tag}{kc}")
            nc.vector.tensor_copy(t_w[:, :], stg_w[:, :])
            acc.append(t_w)

    def r(t):
        return t[:, :]

    # q^T and k^T, packed two heads (128 rows) per pair tile
    qT = []
    kT = []
    for name, wlist, bias_sb, acc in (("qT", wq, bq_sb, qT), ("kT", wk, bk_sb, kT)):
        for pr in range(NPR):
            dst = persist.tile([128, T], F32R, tag=f"{name}{pr}")
            for h2 in range(2):  # halves of T
                ps = psum.tile([128, 1024], F32, tag="sc")
                for kc in range(NKC):
                    for nn in range(2):
                        n0 = (h2 * 2 + nn) * 512
                        nc.tensor.matmul(
                            ps[:, nn * 512:(nn + 1) * 512],
                            r(wlist[kc])[:, pr * 128:(pr + 1) * 128],
                            r(xt[kc])[:, n0:n0 + 512],
                            start=(kc == 0), stop=(kc == NKC - 1),
                        )
                nc.scalar.activation(
                    dst[:, h2 * 1024:(h2 + 1) * 1024], ps[:, :],
                    mybir.ActivationFunctionType.Identity,
                    bias=bias_sb[:, pr:pr + 1], scale=1.0,
                )
            acc.append(dst)

    # v in [token, feature] layout, bf16, bias added
    v_sb = []
    for tt in range(NT):
        psv = psum.tile([128, HW], F32, tag="av")
        for kc in range(NKC):
            nc.tensor.matmul(
                psv[:, :],
                r(xt[kc])[:, tt * 128:(tt + 1) * 128],
                r(wv[kc])[:, :],
                start=(kc == 0), stop=(kc == NKC - 1),
            )
        dst = persist.tile([128, HW], BF16, tag=f"v{tt}")
        nc.vector.tensor_tensor(
            out=dst[:, :], in0=psv[:, :], in1=bv_bc[:, :], op=mybir.AluOpType.add
        )
        v_sb.append(dst)

    # ---- stage 2: attention ----
    ao_sb = [
        persist.tile([128, HW], BF16, tag=f"ao{qt}", name=f"ao{qt}")
        for qt in range(NT)
    ]

    for h in range(HL):
        pr, half = divmod(h, 2)
        p0 = half * 64
        qTh = qT[pr]
        kTh = kT[pr]

        state = {}

        def emit_scores(qt):
            ps_pair = []
            for s2 in range(2):
                ps = psum.tile([128, 1024], F32, tag="sc")
                for nn in range(2):
                    n0 = (s2 * 2 + nn) * 512
                    nc.tensor.matmul(
                        ps[:, nn * 512:(nn + 1) * 512],
                        qTh[p0:p0 + 64, qt * 128:(qt + 1) * 128],
                        kTh[p0:p0 + 64, n0:n0 + 512],
                        start=True, stop=True,
                    )
                ps_pair.append(ps)
            m2 = stats.tile([128, 2], F32, tag="m2")
            nc.vector.reduce_max(m2[:, 0:1], ps_pair[0][:, :], axis=mybir.AxisListType.X)
            nc.vector.reduce_max(m2[:, 1:2], ps_pair[1][:, :], axis=mybir.AxisListType.X)
            nb8 = stats.tile([128, 1], F32, tag="nb8")
            mc = stats.tile([128, 1], F32, tag="mc")
            nc.vector.tensor_tensor(
                out=mc[:, :], in0=m2[:, 0:1], in1=m2[:, 1:2], op=mybir.AluOpType.max
            )
            nc.vector.tensor_scalar_mul(nb8[:, :], mc[:, :], -8.0)
            p_bf = work.tile([128, T], BF16, tag="pbf")
            z2 = stats.tile([128, 2], F32, tag="z2")
            for s2 in range(2):
                nc.scalar.activation(
                    p_bf[:, s2 * 1024:(s2 + 1) * 1024], ps_pair[s2][:, :],
                    mybir.ActivationFunctionType.Exp,
                    bias=nb8[:, 0:1], scale=8.0,
                    accum_out=z2[:, s2:s2 + 1],
                )
            z = stats.tile([128, 1], F32, tag="z")
            nc.vector.tensor_tensor(
                out=z[:, :], in0=z2[:, 0:1], in1=z2[:, 1:2], op=mybir.AluOpType.add
            )
            rz = stats.tile([128, 1], F32, tag="rz")
            nc.vector.reciprocal(rz[:, :], z[:, :])
            state[qt] = (p_bf, rz)

        def emit_consume(qt):
            p_bf, rz = state.pop(qt)
            pT = work.tile([128, T], BF16, tag="pT")
            for grp in range(4):
                pst = psum.tile([128, 512], BF16, tag="tr")
                for j in range(4):
                    c = grp * 4 + j
                    nc.tensor.transpose(
                        pst[:, j * 128:(j + 1) * 128],
                        p_bf[:, c * 128:(c + 1) * 128],
                        ident[:, :],
                    )
                eng = PT_COPY_ENGINE[grp]
                if eng == "vector":
                    nc.vector.tensor_copy(pT[:, grp * 512:(grp + 1) * 512], pst[:, :])
                else:
                    nc.scalar.copy(pT[:, grp * 512:(grp + 1) * 512], pst[:, :])
            pso = psum.tile([128, DH], F32, tag="av")
            for c in range(NT):
                nc.tensor.matmul(
                    pso[:, :],
                    pT[:, c * 128:(c + 1) * 128],
                    v_sb[c][:, h * DH:(h + 1) * DH],
                    start=(c == 0), stop=(c == NT - 1),
                )
            nc.scalar.activation(
                ao_sb[qt][:, h * DH:(h + 1) * DH], pso[:, :],
                mybir.ActivationFunctionType.Copy,
                bias=0.0, scale=rz[:, 0:1],
            )

        for qt in range(NT):
            emit_scores(qt)
            if qt > 0:
                emit_consume(qt - 1)
        emit_consume(NT - 1)

    # ---- stage 3: output projection (partial over local heads) ----
    pjs = []
    for c3 in range(3):
        stg = work.tile([128, C], F32, tag="pstage")
        nc.sync.dma_start(out=stg[:, :], in_=projT[c3 * 128:(c3 + 1) * 128, :])
        pj = const.tile([128, C], BF16, tag=f"pj{c3}")
        nc.vector.tensor_copy(pj[:, :], stg[:, :])
        pjs.append(pj)

    for qt in range(NT):
        pst2 = psum.tile([128, HW], BF16, tag="tr")
        for c3 in range(3):
            nc.tensor.transpose(
                pst2[:, c3 * 128:(c3 + 1) * 128],
                ao_sb[qt][:, c3 * 128:(c3 + 1) * 128],
                ident[:, :],
            )
        aoT = work.tile([128, HW], BF16, tag="aoT")
        nc.vector.tensor_copy(aoT[:, :], pst2[:, :])
        out_sb = work.tile([128, C], F32, tag="outsb")
        for n0, nsz in ((0, 512), (512, 256)):
            psp = psum.tile([128, 512], F32, tag="sc")
            for c3 in range(3):
                nc.tensor.matmul(
                    psp[:, 0:nsz],
                    aoT[:, c3 * 128:(c3 + 1) * 128],
                    pjs[c3][:, n0:n0 + nsz],
                    start=(c3 == 0), stop=(c3 == 2),
                )
            nc.scalar.copy(out_sb[:, n0:n0 + nsz], psp[:, 0:nsz])
        nc.sync.dma_start(out=out[qt * 128:(qt + 1) * 128, :], in_=out_sb[:, :])

    ctx.close()


_NC = None


def _get_nc():
    global _NC
    if _NC is None:
        _NC = build_nc()
    return _NC


def make_core_inputs(x, qkv_w, qkv_b, proj_w, proj_b):
    x = np.asarray(x, dtype=np.float32)
    qkv_w = np.asarray(qkv_w, dtype=np.float32)
    qkv_b = np.asarray(qkv_b, dtype=np.float32)
    proj_w = np.asarray(proj_w, dtype=np.float32)

    ins = []
    for c in range(8):
        b, g = divmod(c, 2)
        hs = slice(g * HW, (g + 1) * HW)
        wq = qkv_w[0:C][hs]
        wk = qkv_w[C:2 * C][hs]
        wv = qkv_w[2 * C:3 * C][hs]
        bq = qkv_b[0:C][hs]
        bk = qkv_b[C:2 * C][hs]
        bvv = qkv_b[2 * C:3 * C][hs]
        ins.append({
            "xT": np.ascontiguousarray(x[b].T),
            "wqT": np.ascontiguousarray(wq.T),
            "wkT": np.ascontiguousarray(wk.T),
            "wvT": np.ascontiguousarray(wv.T),
            "bqt": np.ascontiguousarray(bq.reshape(NPR, 128).T),
            "bkt": np.ascontiguousarray(bk.reshape(NPR, 128).T),
            "bv": np.ascontiguousarray(bvv),
            "projT": np.ascontiguousarray(proj_w[:, hs].T),
        })
    return ins


def kernel(x, qkv_w, qkv_b, proj_w, proj_b):
    ins = make_core_inputs(x, qkv_w, qkv_b, proj_w, proj_b)
    proj_b = np.asarray(proj_b, dtype=np.float32)
    B = np.asarray(x).shape[0]
    res = run_bass_kernel_spmd(_get_nc(), ins, list(range(8)))
    outp = np.empty((B, T, C), np.float32)
    for b in range(B):
        outp[b] = res.results[2 * b]["out"] + res.results[2 * b + 1]["out"] + proj_b
    return outp
